# revision 36
# baseline (speedup 1.0000x reference)
"""Radius-graph kernel (AlphaNet) for 8 TRN2 NeuronCores.

Problem: for each of B=16 images with N=256 atoms in a 12A cubic periodic
box, build the radius-5A graph over the 27 periodic images, keep the 32
nearest neighbors per receiver, and emit:
  dist [B,N,N*27] f32, dvec [B,N,N*27,3] f32, nn [B] int32.

Key insight: 2*RADIUS < CELL_L, so per (i,j) pair at most ONE of the 27
periodic images can be within the radius (per component the viable shift
windows [-5,5], [7,12], [-12,-7] are disjoint).  So the candidate set is
the minimum-image displacement matrix [N,N], and the [N, N*27] output is
a one-hot expansion over the selected image index c = 13 + 9n1+3n2+n3.

Sharding: data-parallel over B; each core handles 2 images.
"""

import numpy as np

import concourse.bass as bass
import concourse.bass_isa as bass_isa
import concourse.tile as tile
from concourse import bacc, mybir
from concourse.bass_utils import run_bass_kernel_spmd

F32 = mybir.dt.float32
BF16 = mybir.dt.bfloat16
AL = mybir.AluOpType
AF = mybir.ActivationFunctionType

B, N, C = 16, 256, 27
M = N * C                   # 6912
NB = 2                      # images per core
NCORES = 8
JCH = 64                    # sources per expansion chunk
NCH = N // JCH              # 4 chunks
W = JCH * C                 # 1728 output cols per chunk
BIG = 1.0e30


def build_nc(reps=1, emit_dma=True, emit_exp=True):
    nc = bacc.Bacc(None)

    pos = nc.declare_dram_parameter("pos", [NB, N, 3], F32, isOutput=False)
    posT = nc.declare_dram_parameter("posT", [NB, 3, N], F32, isOutput=False)
    ccol = nc.declare_dram_parameter("ccol", [128, W], BF16, isOutput=False)
    # packed constants: col 0 = 1.0, cols 1:257 = -BIG, cols 257:513 = 0.0
    cbin = nc.declare_dram_parameter("cb", [128, 513], F32, isOutput=False)
    dist = nc.declare_dram_parameter("dist", [NB, N, M], F32, isOutput=True)
    dvec = nc.declare_dram_parameter("dvec", [NB, N, 3 * M], F32, isOutput=True)
    nnout = nc.declare_dram_parameter("nn", [NB, 1], F32, isOutput=True)

    with tile.TileContext(nc) as tc:
        with (
            tc.tile_pool(name="const", bufs=1) as constp,
            tc.tile_pool(name="src", bufs=2) as srcp,
            tc.tile_pool(name="geom", bufs=2) as geomp,
            tc.tile_pool(name="exp", bufs=2) as expp,
            tc.tile_pool(name="vch", bufs=2) as vchp,
            tc.tile_pool(name="psum", bufs=2, space="PSUM") as psump,
        ):
            ccol_t = constp.tile([128, W], BF16, tag="ccol")
            nc.sync.dma_start(ccol_t[:, :], ccol[:, :])
            cb = constp.tile([128, 513], F32, tag="cb")
            nc.sync.dma_start(cb[:, :], cbin[:, :])
            ones128 = cb[:, 0:1]
            nbig = cb[:, 1:257]
            zero = cb[:, 257:513]

            for b in [b for _ in range(reps) for b in range(NB)]:
                # S[p, k, j] = pos[b, j, k] broadcast to all 128 partitions
                Sf = srcp.tile([128, 3 * N], F32, tag="S")
                for k in range(3):
                    nc.sync.dma_start(
                        Sf[:, k * N : (k + 1) * N],
                        posT[b, k, :].unsqueeze(0).broadcast_to([128, N]),
                    )
                S = Sf[:, :].rearrange("p (k n) -> p k n", k=3, n=N)

                kcnt = geomp.tile([128, 2], F32, tag="kcnt")

                for h in range(2):
                    r0 = h * 128
                    # R[p, k] = pos[b, r0+p, k]
                    R = geomp.tile([128, 3], F32, tag="R")
                    nc.sync.dma_start(R[:, :], pos[b, r0 : r0 + 128, :])

                    g = geomp.tile([128, 3, N], F32, tag="g")
                    da = geomp.tile([128, 3, N], F32, tag="da")
                    m = geomp.tile([128, 3, N], F32, tag="m")
                    q = geomp.tile([128, 3, N], F32, tag="q")
                    d = geomp.tile([128, 3, N], F32, tag="d")
                    gf = g[:, :, :].rearrange("p a b -> p (a b)")
                    daf = da[:, :, :].rearrange("p a b -> p (a b)")
                    mf = m[:, :, :].rearrange("p a b -> p (a b)")
                    qf = q[:, :, :].rearrange("p a b -> p (a b)")
                    Sfl = Sf[:, :]
                    for k in range(3):
                        # g = p_j - p_i   (selection only; gap-protected)
                        nc.vector.tensor_scalar(
                            g[:, k, :], S[:, k, :], R[:, k : k + 1], None, AL.subtract
                        )
                    # k-independent steps merged over [128, 768]
                    nc.vector.tensor_scalar(daf, gf, 6.0, None, AL.is_gt)
                    # m = (g>6) - (g<-6) in {-1,0,1};  image shift n = -m
                    nc.vector.tensor_scalar(qf, gf, -6.0, None, AL.is_lt)
                    nc.vector.tensor_tensor(mf, daf, qf, AL.subtract)
                    # q = p_j + 12n  (= S + m*(-12)), ref rounding order
                    nc.vector.tensor_scalar(qf, mf, -12.0, None, AL.mult)
                    nc.vector.tensor_tensor(qf, Sfl, qf, AL.add)
                    for k in range(3):
                        # d = p_i - q  (= (q * -1) + R), ref rounding order
                        nc.vector.tensor_scalar(
                            d[:, k, :], q[:, k, :], -1.0, R[:, k : k + 1], AL.mult, AL.add
                        )

                    # csel = 13 - 9*m1 - 3*m2 - m3
                    csel = geomp.tile([128, N], F32, tag="csel")
                    t2 = geomp.tile([128, N], F32, tag="t2")
                    nc.vector.tensor_scalar(csel[:, :], m[:, 0, :], -9.0, 13.0, AL.mult, AL.add)
                    nc.vector.tensor_scalar(t2[:, :], m[:, 1, :], -3.0, None, AL.mult)
                    nc.vector.tensor_tensor(csel[:, :], csel[:, :], t2[:, :], AL.add)
                    nc.vector.tensor_tensor(csel[:, :], csel[:, :], m[:, 2, :], AL.subtract)

                    # dsqr, bit-exact mirror of reference
                    sq0 = geomp.tile([128, N], F32, tag="sq0")
                    sq1 = geomp.tile([128, N], F32, tag="sq1")
                    dsqr = geomp.tile([128, N], F32, tag="dsqr")
                    nc.vector.tensor_tensor(sq0[:, :], d[:, 0, :], d[:, 0, :], AL.mult)
                    nc.vector.tensor_tensor(sq1[:, :], d[:, 1, :], d[:, 1, :], AL.mult)
                    nc.vector.tensor_tensor(dsqr[:, :], sq0[:, :], sq1[:, :], AL.add)
                    nc.vector.tensor_tensor(sq0[:, :], d[:, 2, :], d[:, 2, :], AL.mult)
                    nc.vector.tensor_tensor(dsqr[:, :], dsqr[:, :], sq0[:, :], AL.add)

                    # validity: 1e-4 < dsqr <= 25   (uint8 masks for select)
                    u1 = geomp.tile([128, N], mybir.dt.uint8, tag="u1")
                    u = geomp.tile([128, N], mybir.dt.uint8, tag="u")
                    nc.vector.tensor_scalar(u1[:, :], dsqr[:, :], 25.0, None, AL.is_le)
                    nc.vector.tensor_scalar(u[:, :], dsqr[:, :], 1e-4, None, AL.is_gt)
                    nc.vector.tensor_tensor(u[:, :], u[:, :], u1[:, :], AL.logical_and)

                    # v = valid ? -dsqr : -BIG   (negated, for max-based top-k)
                    ndsq = geomp.tile([128, N], F32, tag="ndsq")
                    v = geomp.tile([128, N], F32, tag="v")
                    nc.vector.tensor_scalar(ndsq[:, :], dsqr[:, :], -1.0, None, AL.mult)
                    nc.vector.select(v[:, :], u[:, :], ndsq[:, :], nbig[:, :])

                    # top-32: 4 rounds of (max8, match_replace)
                    mx = geomp.tile([128, 8], F32, tag="mx")
                    for r in range(4):
                        nc.vector.max(mx[:, :], v[:, :])
                        if r < 3:
                            nc.vector.match_replace(v[:, :], mx[:, :], v[:, :], -BIG)

                    # keep = (ndsq >= max(mx[:,7], -25.5)) & valid
                    nthr = geomp.tile([128, 1], F32, tag="nthr")
                    nc.vector.tensor_scalar(nthr[:, :], mx[:, 7:8], -25.5, None, AL.max)
                    k1 = geomp.tile([128, N], mybir.dt.uint8, tag="k1")
                    nc.vector.tensor_scalar(k1[:, :], ndsq[:, :], nthr[:, 0:1], None, AL.is_ge)
                    keep = geomp.tile([128, N], mybir.dt.uint8, tag="keep")
                    nc.vector.tensor_tensor(keep[:, :], k1[:, :], u[:, :], AL.logical_and)

                    # edge count for this receiver tile
                    keepf = geomp.tile([128, N], F32, tag="keepf")
                    nc.vector.tensor_scalar(keepf[:, :], keep[:, :], 1.0, None, AL.mult)
                    nc.vector.tensor_reduce(
                        kcnt[:, h : h + 1], keepf[:, :], mybir.AxisListType.X, AL.add
                    )

                    # sdist = keep ? sqrt(dsqr) : 0
                    sd0 = geomp.tile([128, N], F32, tag="sd0")
                    sdist = geomp.tile([128, N], F32, tag="sdist")
                    nc.vector.select(sd0[:, :], keep[:, :], dsqr[:, :], zero[:, :])
                    nc.scalar.activation(sdist[:, :], sd0[:, :], AF.Sqrt)

                    # cselp = keep ? csel : -BIG  (never matches a column id)
                    cselp = geomp.tile([128, N], F32, tag="cselp")
                    nc.vector.select(cselp[:, :], keep[:, :], csel[:, :], nbig[:, :])

                    # expansion: for each 64-source chunk, build one-hot
                    # planes and stream them out.  mask/dist run as bf16
                    # contiguous TTs (DVE 2x mode); the dist plane is
                    # upcast to f32 by a gpsimd casting DMA.
                    for jc in range(NCH if emit_exp else 0):
                        j0 = jc * JCH
                        # materialize broadcasts as contiguous bf16 (ACT)
                        cselb = expp.tile([128, W], BF16, tag="cselb")
                        nc.scalar.activation(
                            cselb[:, :].rearrange("p (j c) -> p j c", j=JCH, c=C),
                            cselp[:, j0 : j0 + JCH]
                            .unsqueeze(2)
                            .broadcast_to([128, JCH, C]),
                            AF.Copy,
                        )
                        sdb = expp.tile([128, W], BF16, tag="sdb")
                        nc.scalar.activation(
                            sdb[:, :].rearrange("p (j c) -> p j c", j=JCH, c=C),
                            sdist[:, j0 : j0 + JCH]
                            .unsqueeze(2)
                            .broadcast_to([128, JCH, C]),
                            AF.Copy,
                        )
                        m27 = expp.tile([128, W], BF16, tag="m27")
                        nc.vector.tensor_tensor(
                            m27[:, :], cselb[:, :], ccol_t[:, :], AL.is_equal
                        )
                        m27v = m27[:, :].rearrange("p (j c) -> p j c", j=JCH, c=C)

                        dch = expp.tile([128, W], BF16, tag="dch")
                        nc.vector.tensor_tensor(
                            dch[:, :], m27[:, :], sdb[:, :], AL.mult
                        )
                        if emit_dma:
                            nc.gpsimd.dma_start(
                                dist[b, r0 : r0 + 128, jc * W : (jc + 1) * W],
                                dch[:, :],
                            )

                        vch = vchp.tile([128, 3 * W], F32, tag="vch")
                        vchv = vch[:, :].rearrange(
                            "p (j c k) -> p j c k", j=JCH, c=C, k=3
                        )
                        for k in range(3):
                            dkb = (
                                d[:, k, j0 : j0 + JCH]
                                .unsqueeze(2)
                                .broadcast_to([128, JCH, C])
                            )
                            nc.vector.tensor_tensor(vchv[:, :, :, k], m27v, dkb, AL.mult)
                        if emit_dma:
                            nc.sync.dma_start(
                                dvec[b, r0 : r0 + 128, jc * 3 * W : (jc + 1) * 3 * W],
                                vch[:, :],
                            )

                # nn[b] = total kept edges in image b (cross-partition sum
                # via PE: kred[128,1]^T @ ones[128,1])
                kred = geomp.tile([128, 1], F32, tag="kred")
                nc.vector.tensor_reduce(
                    kred[:, :], kcnt[:, :], mybir.AxisListType.X, AL.add
                )
                nnp = psump.tile([1, 1], F32, tag="nnp")
                nc.tensor.matmul(nnp[:, :], kred[:, :], ones128[:, :])
                nnt = geomp.tile([1, 1], F32, tag="nnt")
                nc.scalar.copy(nnt[:, :], nnp[:, :])
                nc.sync.dma_start(nnout[b, 0:1], nnt[:, :])

    nc.finalize()
    return nc


_NC_CACHE = None


def _get_nc():
    global _NC_CACHE
    if _NC_CACHE is None:
        _NC_CACHE = build_nc()
    return _NC_CACHE


def make_in_maps(pos):
    import ml_dtypes

    ccol_np = np.ascontiguousarray(
        np.tile(np.tile(np.arange(C, dtype=np.float32), JCH)[None, :], (128, 1))
    ).astype(ml_dtypes.bfloat16)
    cb_np = np.zeros((128, 513), np.float32)
    cb_np[:, 0] = 1.0
    cb_np[:, 1:257] = -BIG
    in_maps = []
    for c in range(NCORES):
        sh = np.ascontiguousarray(pos[c * NB : (c + 1) * NB])
        shT = np.ascontiguousarray(sh.transpose(0, 2, 1))
        in_maps.append({"pos": sh, "posT": shT, "ccol": ccol_np, "cb": cb_np})
    return in_maps


def run(pos, trace=False):
    pos = np.ascontiguousarray(np.asarray(pos, dtype=np.float32))
    assert pos.shape == (B, N, 3)
    in_maps = make_in_maps(pos)
    nc = _get_nc()
    res = run_bass_kernel_spmd(
        nc, in_maps, core_ids=list(range(NCORES)), trace=trace
    )
    results = res.results
    dist = np.concatenate([r["dist"] for r in results], axis=0)
    dvec = np.concatenate([r["dvec"] for r in results], axis=0).reshape(B, N, M, 3)
    nn = np.concatenate([r["nn"] for r in results], axis=0).reshape(B)
    nn = np.rint(nn).astype(np.int32)
    return (dist, dvec, nn), res.exec_time_ns


def kernel(pos, cell):
    # cell is fixed at 12*I by construction; the kernel hardcodes L=12.
    (dist, dvec, nn), _ = run(pos)
    return dist, dvec, nn


def build_baseline_nc():
    """Same I/O signature, near-zero work: for dispatch-overhead baseline."""
    nc = bacc.Bacc(None)
    nc.declare_dram_parameter("pos", [NB, N, 3], F32, isOutput=False)
    nc.declare_dram_parameter("posT", [NB, 3, N], F32, isOutput=False)
    ccol = nc.declare_dram_parameter("ccol", [128, W], BF16, isOutput=False)
    nc.declare_dram_parameter("cb", [128, 513], F32, isOutput=False)
    dist = nc.declare_dram_parameter("dist", [NB, N, M], F32, isOutput=True)
    dvec = nc.declare_dram_parameter("dvec", [NB, N, 3 * M], F32, isOutput=True)
    nnout = nc.declare_dram_parameter("nn", [NB, 1], F32, isOutput=True)
    with tile.TileContext(nc) as tc:
        with tc.tile_pool(name="p", bufs=1) as p:
            t = p.tile([1, 1], F32, tag="t")
            nc.sync.dma_start(t[:, :], ccol[0:1, 0:1])
            nc.sync.dma_start(dist[0, 0:1, 0:1], t[:, :])
            nc.sync.dma_start(dvec[0, 0:1, 0:1], t[:, :])
            nc.sync.dma_start(nnout[0, 0:1], t[:, :])
    nc.finalize()
    return nc


def _make_runner(nc, in_maps):
    """Compile a non-donating shard_map runner with device-resident args.

    Returns fn() -> jax outputs (device); call block_until_ready yourself.
    """
    import jax
    from jax.sharding import Mesh, NamedSharding, PartitionSpec
    from jax.experimental.shard_map import shard_map
    from concourse import bass2jax
    from concourse.bass2jax import (
        _bass_exec_p,
        install_neuronx_cc_hook,
        partition_id_tensor,
    )

    install_neuronx_cc_hook()
    partition_name = nc.partition_id_tensor.name if nc.partition_id_tensor else None
    in_names, out_names, out_avals, zero_outs = [], [], [], []
    import jax.core
    for alloc in nc.m.functions[0].allocations:
        if not isinstance(alloc, mybir.MemoryLocationSet):
            continue
        name = alloc.memorylocations[0].name
        if alloc.kind == "ExternalInput":
            if name != partition_name:
                in_names.append(name)
        elif alloc.kind == "ExternalOutput":
            out_names.append(name)
            shape = tuple(alloc.tensor_shape)
            dtype = mybir.dt.np(alloc.dtype)
            out_avals.append(jax.core.ShapedArray(shape, dtype))
            zero_outs.append(np.zeros(shape, dtype))
    n_params = len(in_names)
    all_names = in_names + out_names
    if partition_name is not None:
        all_names = all_names + [partition_name]

    def _body(*args):
        operands = list(args)
        if partition_name is not None:
            operands.append(partition_id_tensor())
        outs = _bass_exec_p.bind(
            *operands,
            out_avals=tuple(out_avals),
            in_names=tuple(all_names),
            out_names=tuple(out_names),
            lowering_input_output_aliases=(),
            sim_require_finite=False,
            sim_require_nnan=False,
            nc=nc,
        )
        return tuple(outs)

    devices = jax.devices()[:NCORES]
    mesh = Mesh(np.asarray(devices), ("core",))
    nin = n_params + len(out_names)
    fn = jax.jit(
        shard_map(
            _body,
            mesh=mesh,
            in_specs=(PartitionSpec("core"),) * nin,
            out_specs=(PartitionSpec("core"),) * len(out_names),
            check_rep=False,
        ),
        keep_unused=True,
    )
    per_core = [[np.asarray(m[k]) for k in in_names] for m in in_maps]
    concat_in = [
        np.concatenate([per_core[c][i] for c in range(NCORES)], axis=0)
        for i in range(n_params)
    ]
    concat_zeros = [
        np.zeros((NCORES * z.shape[0], *z.shape[1:]), z.dtype) for z in zero_outs
    ]
    sh = NamedSharding(mesh, PartitionSpec("core"))
    dev_args = [jax.device_put(a, sh) for a in concat_in + concat_zeros]
    return lambda: fn(*dev_args)


def bench(pos, iters=30):
    import time as _time
    import jax

    pos = np.ascontiguousarray(np.asarray(pos, dtype=np.float32))
    in_maps = make_in_maps(pos)

    def timeloop(fn):
        out = fn()
        jax.block_until_ready(out)  # compile+warm
        out = fn()
        jax.block_until_ready(out)
        ts = []
        for _ in range(iters):
            t0 = _time.perf_counter()
            out = fn()
            jax.block_until_ready(out)
            ts.append(_time.perf_counter() - t0)
        ts.sort()
        return ts

    KREP = 8
    fn1 = _make_runner(_get_nc(), in_maps)
    ts1 = timeloop(fn1)
    fnK = _make_runner(build_nc(reps=KREP), in_maps)
    tsK = timeloop(fnK)

    med = lambda ts: ts[len(ts) // 2]
    exec_ns = (tsK[0] - ts1[0]) / (KREP - 1) * 1e9
    stats = {
        "t1_min_ms": ts1[0] * 1e3,
        "t1_median_ms": med(ts1) * 1e3,
        "tK_min_ms": tsK[0] * 1e3,
        "tK_median_ms": med(tsK) * 1e3,
        "exec_ns_min_diff": exec_ns,
        "exec_ns_median_diff": (med(tsK) - med(ts1)) / (KREP - 1) * 1e9,
    }
    return exec_ns, stats


# revision 58
# speedup vs baseline: 72.8628x; 72.8628x over previous
"""Radius-graph kernel (AlphaNet) for 8 TRN2 NeuronCores.

Problem: for each of B=16 images with N=256 atoms in a 12A cubic periodic
box, build the radius-5A graph over the 27 periodic images, keep the 32
nearest neighbors per receiver, and emit:
  dist [B,N,N*27] f32, dvec [B,N,N*27,3] f32, nn [B] int32.

Key insight: 2*RADIUS < CELL_L, so per (i,j) pair at most ONE of the 27
periodic images can be within the radius (per component the viable shift
windows [-5,5], [7,12], [-12,-7] are disjoint).  So the candidate set is
the minimum-image displacement matrix [N,N], and the [N, N*27] output is
a one-hot expansion over the selected image index c = 13 + 9n1+3n2+n3.

Sharding: data-parallel over B; each core handles 2 images.
"""

import numpy as np

import concourse.bass as bass
import concourse.bass_isa as bass_isa
import concourse.tile as tile
from concourse import bacc, mybir
from concourse.bass_utils import run_bass_kernel_spmd

F32 = mybir.dt.float32
BF16 = mybir.dt.bfloat16
AL = mybir.AluOpType
AF = mybir.ActivationFunctionType

B, N, C = 16, 256, 27
M = N * C                   # 6912
NB = 2                      # images per core
NCORES = 8
JCH = 64                    # sources per expansion chunk
NCH = N // JCH              # 4 chunks
W = JCH * C                 # 1728 output cols per chunk
BIG = 1.0e30


def build_nc(
    reps=1, emit_dma=True, emit_exp=True, pool_ilv=True, emit_expc=True, dup=""
):
    nc = bacc.Bacc(None)

    pos = nc.declare_dram_parameter("pos", [NB, N, 3], F32, isOutput=False)
    posT = nc.declare_dram_parameter("posT", [NB, 3, N], F32, isOutput=False)
    ccol = nc.declare_dram_parameter("ccol", [128, W], BF16, isOutput=False)
    # packed constants: col 0 = 1.0, cols 1:257 = -BIG, cols 257:513 = 0.0
    cbin = nc.declare_dram_parameter("cb", [128, 513], F32, isOutput=False)
    dist = nc.declare_dram_parameter("dist", [NB, N, M], F32, isOutput=True)
    dvec = nc.declare_dram_parameter("dvec", [NB, N, 3 * M], F32, isOutput=True)
    nnout = nc.declare_dram_parameter("nn", [NB, 1], F32, isOutput=True)

    with tile.TileContext(nc) as tc:
        with (
            tc.tile_pool(name="const", bufs=1) as constp,
            tc.tile_pool(name="src", bufs=2) as srcp,
            tc.tile_pool(name="geom", bufs=3) as geomp,
            tc.tile_pool(name="exp", bufs=2) as expp,
            tc.tile_pool(name="vch", bufs=2) as vchp,
            tc.tile_pool(name="psum", bufs=2, space="PSUM") as psump,
        ):
            ccol_t = constp.tile([128, W], BF16, tag="ccol")
            nc.sync.dma_start(ccol_t[:, :], ccol[:, :])
            cb = constp.tile([128, 513], F32, tag="cb")
            nc.sync.dma_start(cb[:, :], cbin[:, :])
            ones128 = cb[:, 0:1]
            nbig = cb[:, 1:257]
            zero = cb[:, 257:513]

            for b in [b for _ in range(reps) for b in range(NB)]:
                # S[p, k, j] = pos[b, j, k] broadcast to all 128 partitions
                Sf = srcp.tile([128, 3 * N], F32, tag="S")
                for k in range(3):
                    nc.sync.dma_start(
                        Sf[:, k * N : (k + 1) * N],
                        posT[b, k, :].unsqueeze(0).broadcast_to([128, N]),
                    )
                S = Sf[:, :].rearrange("p (k n) -> p k n", k=3, n=N)

                kcnt = geomp.tile([128, 2], F32, tag="kcnt")

                for h in range(2):
                    r0 = h * 128
                    # R[p, k] = pos[b, r0+p, k]
                    R = geomp.tile([128, 3], F32, tag="R")
                    nc.sync.dma_start(R[:, :], pos[b, r0 : r0 + 128, :])

                    g = geomp.tile([128, 3, N], F32, tag="g")
                    da = geomp.tile([128, 3, N], F32, tag="da")
                    m = geomp.tile([128, 3, N], F32, tag="m")
                    q = geomp.tile([128, 3, N], F32, tag="q")
                    d = geomp.tile([128, 3, N], F32, tag="d")
                    gf = g[:, :, :].rearrange("p a b -> p (a b)")
                    daf = da[:, :, :].rearrange("p a b -> p (a b)")
                    mf = m[:, :, :].rearrange("p a b -> p (a b)")
                    qf = q[:, :, :].rearrange("p a b -> p (a b)")
                    Sfl = Sf[:, :]
                    for k in range(3):
                        # g = p_j - p_i   (selection only; gap-protected)
                        nc.vector.tensor_scalar(
                            g[:, k, :], S[:, k, :], R[:, k : k + 1], None, AL.subtract
                        )
                    # k-independent steps merged over [128, 768]
                    nc.vector.tensor_scalar(daf, gf, 6.0, None, AL.is_gt)
                    # m = (g>6) - (g<-6) in {-1,0,1};  image shift n = -m
                    nc.vector.tensor_scalar(qf, gf, -6.0, None, AL.is_lt)
                    nc.vector.tensor_tensor(mf, daf, qf, AL.subtract)
                    # q = p_j + 12n  (= S + m*(-12)), ref rounding order
                    nc.vector.tensor_scalar(qf, mf, -12.0, None, AL.mult)
                    nc.vector.tensor_tensor(qf, Sfl, qf, AL.add)
                    for k in range(3):
                        # d = p_i - q  (= (q * -1) + R), ref rounding order
                        nc.vector.tensor_scalar(
                            d[:, k, :], q[:, k, :], -1.0, R[:, k : k + 1], AL.mult, AL.add
                        )

                    # csel = 13 - 9*m1 - 3*m2 - m3
                    csel = geomp.tile([128, N], F32, tag="csel")
                    t2 = geomp.tile([128, N], F32, tag="t2")
                    nc.vector.tensor_scalar(csel[:, :], m[:, 0, :], -9.0, 13.0, AL.mult, AL.add)
                    nc.vector.tensor_scalar(t2[:, :], m[:, 1, :], -3.0, None, AL.mult)
                    nc.vector.tensor_tensor(csel[:, :], csel[:, :], t2[:, :], AL.add)
                    nc.vector.tensor_tensor(csel[:, :], csel[:, :], m[:, 2, :], AL.subtract)

                    # dsqr, bit-exact mirror of reference
                    sq0 = geomp.tile([128, N], F32, tag="sq0")
                    sq1 = geomp.tile([128, N], F32, tag="sq1")
                    dsqr = geomp.tile([128, N], F32, tag="dsqr")
                    nc.vector.tensor_tensor(sq0[:, :], d[:, 0, :], d[:, 0, :], AL.mult)
                    nc.vector.tensor_tensor(sq1[:, :], d[:, 1, :], d[:, 1, :], AL.mult)
                    nc.vector.tensor_tensor(dsqr[:, :], sq0[:, :], sq1[:, :], AL.add)
                    nc.vector.tensor_tensor(sq0[:, :], d[:, 2, :], d[:, 2, :], AL.mult)
                    nc.vector.tensor_tensor(dsqr[:, :], dsqr[:, :], sq0[:, :], AL.add)

                    # validity: 1e-4 < dsqr <= 25   (uint8 masks for select)
                    u1 = geomp.tile([128, N], mybir.dt.uint8, tag="u1")
                    u = geomp.tile([128, N], mybir.dt.uint8, tag="u")
                    nc.vector.tensor_scalar(u1[:, :], dsqr[:, :], 25.0, None, AL.is_le)
                    nc.vector.tensor_scalar(u[:, :], dsqr[:, :], 1e-4, None, AL.is_gt)
                    nc.vector.tensor_tensor(u[:, :], u[:, :], u1[:, :], AL.logical_and)

                    # v = valid ? -dsqr : -BIG   (negated, for max-based top-k)
                    ndsq = geomp.tile([128, N], F32, tag="ndsq")
                    v = geomp.tile([128, N], F32, tag="v")
                    nc.vector.tensor_scalar(ndsq[:, :], dsqr[:, :], -1.0, None, AL.mult)
                    nc.vector.select(v[:, :], u[:, :], ndsq[:, :], nbig[:, :])

                    # top-32: 4 rounds of (max8, match_replace); v preserved
                    mx = geomp.tile([128, 8], F32, tag="mx")
                    vw = geomp.tile([128, N], F32, tag="vw")
                    for r in range(4):
                        nc.vector.max(mx[:, :], (v if r == 0 else vw)[:, :])
                        if r < 3:
                            nc.vector.match_replace(
                                vw[:, :], mx[:, :], (v if r == 0 else vw)[:, :], -BIG
                            )

                    # keep = (v >= max(mx[:,7], -25.5)); invalid v = -BIG fails
                    nthr = geomp.tile([128, 1], F32, tag="nthr")
                    nc.vector.tensor_scalar(nthr[:, :], mx[:, 7:8], -25.5, None, AL.max)
                    keep = geomp.tile([128, N], mybir.dt.uint8, tag="keep")
                    nc.vector.tensor_scalar(keep[:, :], v[:, :], nthr[:, 0:1], None, AL.is_ge)

                    # edge count for this receiver tile
                    keepf = geomp.tile([128, N], F32, tag="keepf")
                    nc.vector.tensor_scalar(keepf[:, :], keep[:, :], 1.0, None, AL.mult)
                    nc.vector.tensor_reduce(
                        kcnt[:, h : h + 1], keepf[:, :], mybir.AxisListType.X, AL.add
                    )

                    # sdist = sqrt(dsqr); non-kept entries are masked by m27
                    sdist = geomp.tile([128, N], F32, tag="sdist")
                    nc.scalar.activation(sdist[:, :], dsqr[:, :], AF.Sqrt)

                    # cselp = keep ? csel : -BIG  (never matches a column id)
                    cselp = geomp.tile([128, N], F32, tag="cselp")
                    nc.vector.select(cselp[:, :], keep[:, :], csel[:, :], nbig[:, :])

                    # expansion: for each 64-source chunk, build one-hot
                    # planes and stream them out.  mask/dist run as bf16
                    # contiguous TTs (DVE 2x mode); the dist plane is
                    # upcast to f32 by a gpsimd casting DMA.
                    for jc in range(NCH if emit_exp else 0):
                        j0 = jc * JCH
                        # materialize broadcasts as contiguous bf16 (ACT)
                        cselb = expp.tile([128, W], BF16, tag="cselb")
                        nc.scalar.activation(
                            cselb[:, :].rearrange("p (j c) -> p j c", j=JCH, c=C),
                            cselp[:, j0 : j0 + JCH]
                            .unsqueeze(2)
                            .broadcast_to([128, JCH, C]),
                            AF.Copy,
                        )
                        sdb = expp.tile([128, W], BF16, tag="sdb")
                        nc.scalar.activation(
                            sdb[:, :].rearrange("p (j c) -> p j c", j=JCH, c=C),
                            sdist[:, j0 : j0 + JCH]
                            .unsqueeze(2)
                            .broadcast_to([128, JCH, C]),
                            AF.Copy,
                        )
                        m27 = expp.tile([128, W], BF16, tag="m27")
                        if emit_expc:
                            nc.vector.tensor_tensor(
                                m27[:, :], cselb[:, :], ccol_t[:, :], AL.is_equal
                            )
                        m27v = m27[:, :].rearrange("p (j c) -> p j c", j=JCH, c=C)

                        dch = expp.tile([128, W], BF16, tag="dch")
                        if emit_expc:
                            nc.vector.tensor_tensor(
                                dch[:, :], m27[:, :], sdb[:, :], AL.mult
                            )
                        if emit_dma:
                            nc.gpsimd.dma_start(
                                dist[b, r0 : r0 + 128, jc * W : (jc + 1) * W],
                                dch[:, :],
                            )

                        vch = vchp.tile([128, 3 * W], F32, tag="vch")
                        vchv = vch[:, :].rearrange(
                            "p (j c k) -> p j c k", j=JCH, c=C, k=3
                        )
                        if pool_ilv:
                            # vb_k: contiguous bf16 broadcasts (ACT/Pool),
                            # bf16 2x muls on DVE, Pool interleave-casts
                            for k in range(3):
                                vb = expp.tile([128, W], BF16, tag=f"vb{k}")
                                vbv = vb[:, :].rearrange(
                                    "p (j c) -> p j c", j=JCH, c=C
                                )
                                dkb = (
                                    d[:, k, j0 : j0 + JCH]
                                    .unsqueeze(2)
                                    .broadcast_to([128, JCH, C])
                                )
                                if k == 2:
                                    nc.gpsimd.tensor_copy(vbv, dkb)
                                else:
                                    nc.scalar.activation(vbv, dkb, AF.Copy)
                                pk = expp.tile([128, W], BF16, tag=f"pk{k}")
                                if emit_expc:
                                    nc.vector.tensor_tensor(
                                        pk[:, :], m27[:, :], vb[:, :], AL.mult
                                    )
                                    if "dve" in dup:
                                        nc.vector.tensor_tensor(
                                            pk[:, :], m27[:, :], vb[:, :], AL.mult
                                        )
                                if "act" in dup:
                                    nc.scalar.activation(
                                        pk[:, :].rearrange(
                                            "p (j c) -> p j c", j=JCH, c=C
                                        ),
                                        dkb,
                                        AF.Copy,
                                    )
                                nc.gpsimd.tensor_copy(
                                    vchv[:, :, :, k],
                                    pk[:, :].rearrange(
                                        "p (j c) -> p j c", j=JCH, c=C
                                    ),
                                )
                                if "pool" in dup:
                                    nc.gpsimd.tensor_copy(
                                        vchv[:, :, :, k],
                                        pk[:, :].rearrange(
                                            "p (j c) -> p j c", j=JCH, c=C
                                        ),
                                    )
                        else:
                            for k in range(3):
                                dkb = (
                                    d[:, k, j0 : j0 + JCH]
                                    .unsqueeze(2)
                                    .broadcast_to([128, JCH, C])
                                )
                                if emit_expc:
                                    nc.vector.tensor_tensor(
                                        vchv[:, :, :, k], m27v, dkb, AL.mult
                                    )
                        if emit_dma:
                            deng = nc.sync
                            deng.dma_start(
                                dvec[b, r0 : r0 + 128, jc * 3 * W : (jc + 1) * 3 * W],
                                vch[:, :],
                            )

                # nn[b] = total kept edges in image b (cross-partition sum
                # via PE: kred[128,1]^T @ ones[128,1])
                kred = geomp.tile([128, 1], F32, tag="kred")
                nc.vector.tensor_reduce(
                    kred[:, :], kcnt[:, :], mybir.AxisListType.X, AL.add
                )
                nnp = psump.tile([1, 1], F32, tag="nnp")
                nc.tensor.matmul(nnp[:, :], kred[:, :], ones128[:, :])
                nnt = geomp.tile([1, 1], F32, tag="nnt")
                nc.scalar.copy(nnt[:, :], nnp[:, :])
                nc.sync.dma_start(nnout[b, 0:1], nnt[:, :])

    nc.finalize()
    return nc


_NC_CACHE = None


def _get_nc():
    global _NC_CACHE
    if _NC_CACHE is None:
        _NC_CACHE = build_nc()
    return _NC_CACHE


def make_in_maps(pos):
    import ml_dtypes

    ccol_np = np.ascontiguousarray(
        np.tile(np.tile(np.arange(C, dtype=np.float32), JCH)[None, :], (128, 1))
    ).astype(ml_dtypes.bfloat16)
    cb_np = np.zeros((128, 513), np.float32)
    cb_np[:, 0] = 1.0
    cb_np[:, 1:257] = -BIG
    in_maps = []
    for c in range(NCORES):
        sh = np.ascontiguousarray(pos[c * NB : (c + 1) * NB])
        shT = np.ascontiguousarray(sh.transpose(0, 2, 1))
        in_maps.append({"pos": sh, "posT": shT, "ccol": ccol_np, "cb": cb_np})
    return in_maps


def run(pos, trace=False):
    pos = np.ascontiguousarray(np.asarray(pos, dtype=np.float32))
    assert pos.shape == (B, N, 3)
    in_maps = make_in_maps(pos)
    nc = _get_nc()
    res = run_bass_kernel_spmd(
        nc, in_maps, core_ids=list(range(NCORES)), trace=trace
    )
    results = res.results
    dist = np.concatenate([r["dist"] for r in results], axis=0)
    dvec = np.concatenate([r["dvec"] for r in results], axis=0).reshape(B, N, M, 3)
    nn = np.concatenate([r["nn"] for r in results], axis=0).reshape(B)
    nn = np.rint(nn).astype(np.int32)
    return (dist, dvec, nn), res.exec_time_ns


def kernel(pos, cell):
    # cell is fixed at 12*I by construction; the kernel hardcodes L=12.
    (dist, dvec, nn), _ = run(pos)
    return dist, dvec, nn


def build_baseline_nc():
    """Same I/O signature, near-zero work: for dispatch-overhead baseline."""
    nc = bacc.Bacc(None)
    nc.declare_dram_parameter("pos", [NB, N, 3], F32, isOutput=False)
    nc.declare_dram_parameter("posT", [NB, 3, N], F32, isOutput=False)
    ccol = nc.declare_dram_parameter("ccol", [128, W], BF16, isOutput=False)
    nc.declare_dram_parameter("cb", [128, 513], F32, isOutput=False)
    dist = nc.declare_dram_parameter("dist", [NB, N, M], F32, isOutput=True)
    dvec = nc.declare_dram_parameter("dvec", [NB, N, 3 * M], F32, isOutput=True)
    nnout = nc.declare_dram_parameter("nn", [NB, 1], F32, isOutput=True)
    with tile.TileContext(nc) as tc:
        with tc.tile_pool(name="p", bufs=1) as p:
            t = p.tile([1, 1], F32, tag="t")
            nc.sync.dma_start(t[:, :], ccol[0:1, 0:1])
            nc.sync.dma_start(dist[0, 0:1, 0:1], t[:, :])
            nc.sync.dma_start(dvec[0, 0:1, 0:1], t[:, :])
            nc.sync.dma_start(nnout[0, 0:1], t[:, :])
    nc.finalize()
    return nc


def _make_runner(nc, in_maps):
    """Compile a non-donating shard_map runner with device-resident args.

    Returns fn() -> jax outputs (device); call block_until_ready yourself.
    """
    import jax
    from jax.sharding import Mesh, NamedSharding, PartitionSpec
    from jax.experimental.shard_map import shard_map
    from concourse import bass2jax
    from concourse.bass2jax import (
        _bass_exec_p,
        install_neuronx_cc_hook,
        partition_id_tensor,
    )

    install_neuronx_cc_hook()
    partition_name = nc.partition_id_tensor.name if nc.partition_id_tensor else None
    in_names, out_names, out_avals, zero_outs = [], [], [], []
    import jax.core
    for alloc in nc.m.functions[0].allocations:
        if not isinstance(alloc, mybir.MemoryLocationSet):
            continue
        name = alloc.memorylocations[0].name
        if alloc.kind == "ExternalInput":
            if name != partition_name:
                in_names.append(name)
        elif alloc.kind == "ExternalOutput":
            out_names.append(name)
            shape = tuple(alloc.tensor_shape)
            dtype = mybir.dt.np(alloc.dtype)
            out_avals.append(jax.core.ShapedArray(shape, dtype))
            zero_outs.append(np.zeros(shape, dtype))
    n_params = len(in_names)
    all_names = in_names + out_names
    if partition_name is not None:
        all_names = all_names + [partition_name]

    def _body(*args):
        operands = list(args)
        if partition_name is not None:
            operands.append(partition_id_tensor())
        outs = _bass_exec_p.bind(
            *operands,
            out_avals=tuple(out_avals),
            in_names=tuple(all_names),
            out_names=tuple(out_names),
            lowering_input_output_aliases=(),
            sim_require_finite=False,
            sim_require_nnan=False,
            nc=nc,
        )
        return tuple(outs)

    devices = jax.devices()[:NCORES]
    mesh = Mesh(np.asarray(devices), ("core",))
    nin = n_params + len(out_names)
    fn = jax.jit(
        shard_map(
            _body,
            mesh=mesh,
            in_specs=(PartitionSpec("core"),) * nin,
            out_specs=(PartitionSpec("core"),) * len(out_names),
            check_rep=False,
        ),
        keep_unused=True,
    )
    per_core = [[np.asarray(m[k]) for k in in_names] for m in in_maps]
    concat_in = [
        np.concatenate([per_core[c][i] for c in range(NCORES)], axis=0)
        for i in range(n_params)
    ]
    concat_zeros = [
        np.zeros((NCORES * z.shape[0], *z.shape[1:]), z.dtype) for z in zero_outs
    ]
    sh = NamedSharding(mesh, PartitionSpec("core"))
    dev_args = [jax.device_put(a, sh) for a in concat_in + concat_zeros]
    return lambda: fn(*dev_args)


def bench(pos, iters=30):
    import time as _time
    import jax

    pos = np.ascontiguousarray(np.asarray(pos, dtype=np.float32))
    in_maps = make_in_maps(pos)

    def timeloop(fn):
        out = fn()
        jax.block_until_ready(out)  # compile+warm
        out = fn()
        jax.block_until_ready(out)
        ts = []
        for _ in range(iters):
            t0 = _time.perf_counter()
            out = fn()
            jax.block_until_ready(out)
            ts.append(_time.perf_counter() - t0)
        ts.sort()
        return ts

    KREP = 8
    fn1 = _make_runner(_get_nc(), in_maps)
    ts1 = timeloop(fn1)
    fnK = _make_runner(build_nc(reps=KREP), in_maps)
    tsK = timeloop(fnK)

    med = lambda ts: ts[len(ts) // 2]
    exec_ns = (tsK[0] - ts1[0]) / (KREP - 1) * 1e9
    stats = {
        "t1_min_ms": ts1[0] * 1e3,
        "t1_median_ms": med(ts1) * 1e3,
        "tK_min_ms": tsK[0] * 1e3,
        "tK_median_ms": med(tsK) * 1e3,
        "exec_ns_min_diff": exec_ns,
        "exec_ns_median_diff": (med(tsK) - med(ts1)) / (KREP - 1) * 1e9,
    }
    return exec_ns, stats
